# revision 41
# baseline (speedup 1.0000x reference)
"""Trainium2 Bass kernel for nn_Loss_factory_12429635355015.

Loss = NLLSurv + CohortLoss(intra + inter) over a [4, 8192, 4, 256] cohort bank.

Strategy (memory-bound, 8 NeuronCores):
  - Shard cohort_bank along the N (bank-entry) axis: each core streams its
    16 MiB shard once at HBM line rate (8 tiles x 2 MiB contiguous SWDGE
    cast-DMAs, f32 HBM -> bf16 SBUF).
  - Per 512-entry tile the compute is balanced so every engine stays under
    the ~5.9us/tile DMA floor:
      DVE:  2 adds for the 4-component sum + segmented reduce for ||S||^2.
      PE :  8 transposes, then the anchor matmul in TRANSPOSED orientation:
            P_T[n, b] = st.T @ at, so the out-partition of block e is the
            original bank partition p.
      ACT:  Square; rh4 = exp(-0.5 ln ssq + ln .5) = 0.5/||S||; then
            Exp(P_T, scale=rh4[:, e]) fuses the l2-norm AND 1/tau into the
            exp's per-partition scale. Per-class sums are PE ones-matmuls
            accumulated in PSUM across each class's tiles.
  - NLL + intra terms are computed on-device from host-encoded one-hots
    (index encoding only; all arithmetic on device).
  - Each core outputs [ep_partial, en_partial, nll+intra]; the host sums the
    two scalars across cores (the 'all-reduce two scalars' step) and applies
    the final -log((ep+eps)/(ep+en+eps)).
"""

import math
import os
import sys

import numpy as np

for _p in ("/opt/trn_rl_repo",):
    if _p not in sys.path and os.path.isdir(_p):
        sys.path.insert(0, _p)

import concourse.bacc as bacc
import concourse.tile as tile
from concourse import mybir
from concourse.bass_utils import run_bass_kernel_spmd

# Pin every activation to the one table set that contains all functions this
# kernel uses (Square/Ln/Exp/Copy/Abs/Identity). Without this, Bacc's
# first-match set selection alternates between sets (Ln lives outside the
# default exp set) and reloads the ACT tables ~1.3us per switch every tile.
_ACT_SET = "natural_log_exp_and_others"


def _pin_act_tables():
    import functools
    import concourse.hw_specs as hw_specs
    if getattr(hw_specs.get_activation_tables, "_pinned", False):
        return
    orig = hw_specs.get_activation_tables

    @functools.cache
    def pinned(arch):
        tabs = orig(arch)
        return {k: (v if k == _ACT_SET else set()) for k, v in tabs.items()}

    pinned._pinned = True
    hw_specs.get_activation_tables = pinned
    bacc.get_activation_tables = pinned


_pin_act_tables()

F32 = mybir.dt.float32
AF = mybir.ActivationFunctionType

# Problem constants (hardcoded per spec).
B = 64            # batch
K = 4             # n_cls
C = 256           # feature dim
NB = 8192         # bank entries per class (global)
NCORES = 8
NSH = NB // NCORES          # 1024 bank entries per class per core
ROWS = K * NSH              # 4096 rows of [4*256] per core
NT = 512                    # bank entries per tile (2 MiB)
TILES = ROWS // NT          # 8
TILES_PER_CLASS = NSH // NT # 2
EPG = NT // 128             # 4 entries per partition per tile
LN_HALF = math.log(0.5)
EPS_NLL = 1e-7
EPS_COH = 1e-8


def _build():
    nc = bacc.Bacc("TRN2", target_bir_lowering=False, debug=False,
                   enable_asserts=False, num_devices=NCORES)

    # All small inputs ride in ONE blob DMA so the prologue is never starved
    # behind the saturated bank stream (13 small HWDGE DMAs serialized for
    # ~25us otherwise).  Layout (f32, [128, BLOBW]):
    #   [0:64,    0:1024] indiv      [0:64, 1024:1536] gene|path
    #   [0:64, 1536:1540] hazards    [0:64, 1540:1545] S_padded
    #   [0:64, 1545:1550] onehot(Y)  [0:64, 1550:1555] onehot(Y+1)
    #   [0:64, 1555:1557] [cf, 1-cf] [0:1,  1557:1813] class-of-b mask row
    BLOBW = 1824
    OFF_GP, OFF_HAZ, OFF_SPAD = 1024, 1536, 1540
    OFF_OHY, OFF_OHY1, OFF_CFS, OFF_OH4T = 1545, 1550, 1555, 1557
    bank = nc.dram_tensor("bank", [ROWS, 1024], F32, kind="ExternalInput")
    blob = nc.dram_tensor("blob", [128, BLOBW], F32, kind="ExternalInput")

    out_d = nc.dram_tensor("out_vec", [1, 4], F32, kind="ExternalOutput")
    e_dbg = nc.dram_tensor("e_dbg", [1, K * B], F32, kind="ExternalOutput")

    import ml_dtypes
    cst_f = np.zeros((128, 129), dtype=np.float32)
    cst_f[:, 0:128] = np.eye(128, dtype=np.float32)
    cst_f[:, 128] = 1.0
    cst_b = np.zeros((128, 129), dtype=ml_dtypes.bfloat16)
    cst_b[:, 0:128] = np.eye(128, dtype=ml_dtypes.bfloat16)
    cst_b[:, 128] = 1.0
    cst_f_d = nc.inline_tensor(cst_f, "cst_f")
    cst_b_d = nc.inline_tensor(cst_b, "cst_b")

    v = nc.vector
    a = nc.scalar

    with tile.TileContext(nc) as tc:
        from contextlib import ExitStack
        with ExitStack() as ctx:
            const = ctx.enter_context(tc.tile_pool(name="const", bufs=1))
            small = ctx.enter_context(tc.tile_pool(name="small", bufs=1))
            tpool = ctx.enter_context(tc.tile_pool(name="T", bufs=3))
            upool = ctx.enter_context(tc.tile_pool(name="U", bufs=2))
            spool = ctx.enter_context(tc.tile_pool(name="S", bufs=2))
            sqpool = ctx.enter_context(tc.tile_pool(name="sq", bufs=2))
            stpool = ctx.enter_context(tc.tile_pool(name="STsb", bufs=2))
            epool = ctx.enter_context(tc.tile_pool(name="esb", bufs=2))
            ps_st = ctx.enter_context(tc.tile_pool(name="ps_st", bufs=2, space="PSUM"))
            ps_p = ctx.enter_context(tc.tile_pool(name="ps_p", bufs=2, space="PSUM"))
            ps_one = ctx.enter_context(tc.tile_pool(name="ps_one", bufs=1, space="PSUM"))

            BF16 = mybir.dt.bfloat16
            # The blob goes first on the SWDGE queue (1us of stream time,
            # needed for the anchors), with bank tiles 0/1 right behind so
            # the big stream starts immediately after.
            blob_sb = small.tile([128, BLOBW], F32)
            nc.gpsimd.dma_start(out=blob_sb[:], in_=blob[:])
            T_tiles = []
            for t in range(2):
                T_sb = tpool.tile([128, 4096], BF16, name="T_sb", tag="T")
                nc.gpsimd.dma_start(
                    out=T_sb.rearrange("p (e x) -> p e x", e=EPG),
                    in_=bank[t * NT:(t + 1) * NT, :].rearrange(
                        "(p e) x -> p e x", e=EPG))
                T_tiles.append(T_sb)
            cst_f_sb = const.tile([128, 129], F32)
            nc.sync.dma_start(out=cst_f_sb[:], in_=cst_f_d[:])
            cst_b_sb = const.tile([128, 129], BF16)
            nc.sync.dma_start(out=cst_b_sb[:], in_=cst_b_d[:])
            ident_sb = cst_f_sb[:, 0:128]
            ones_sb = cst_f_sb[:, 128:129]
            ident_bf = cst_b_sb[:, 0:128]
            ones_bf = cst_b_sb[:, 128:129]
            lnhalf_sb = const.tile([128, 1], F32)
            v.memset(lnhalf_sb[:], LN_HALF)

            # ---------- anchors: A = l2norm(mean_j indiv[b,j,:]) ----------
            ind_sb = blob_sb[0:B, 0:1024]
            iv = ind_sb.rearrange("p (j c) -> p j c", j=4)
            asum = small.tile([B, C], F32)
            atmp = small.tile([B, C], F32)
            v.tensor_add(asum[:], iv[:, 0, :], iv[:, 1, :])
            v.tensor_add(atmp[:], iv[:, 2, :], iv[:, 3, :])
            v.tensor_add(asum[:], asum[:], atmp[:])
            sqa = small.tile([B, C], F32)
            ssa = small.tile([B, 1], F32)
            a.activation(sqa[:], asum[:], AF.Square, accum_out=ssa[:])
            lna = small.tile([B, 1], F32)
            a.activation(lna[:], ssa[:], AF.Ln)
            rsa = small.tile([B, 1], F32)
            a.activation(rsa[:], lna[:], AF.Exp, scale=-0.5)
            v.tensor_scalar_mul(asum[:], asum[:], rsa[:])
            at_ps = ps_one.tile([128, 2, B], F32, tag="at")
            for h in range(2):
                nc.tensor.transpose(at_ps[:, h, :], asum[:, h * 128:(h + 1) * 128],
                                    ident_sb[0:B, 0:B])
            at_sb = const.tile([128, 2, B], BF16)
            a.copy(at_sb[:], at_ps[:])

            # ---------- NLL (per-b, b on partitions) ----------
            haz_sb = blob_sb[0:B, OFF_HAZ:OFF_HAZ + K]
            spad_sb = blob_sb[0:B, OFF_SPAD:OFF_SPAD + K + 1]
            ohy_sb = blob_sb[0:B, OFF_OHY:OFF_OHY + K + 1]
            ohy1_sb = blob_sb[0:B, OFF_OHY1:OFF_OHY1 + K + 1]
            oh4t_sb = blob_sb[0:1, OFF_OH4T:OFF_OH4T + K * B]
            cfs_sb = blob_sb[0:B, OFF_CFS:OFF_CFS + 2]

            t5 = small.tile([B, K + 1], F32)
            t4 = small.tile([B, K], F32)
            sy = small.tile([B, 1], F32)
            hy = small.tile([B, 1], F32)
            sy1 = small.tile([B, 1], F32)
            _stt = dict(op0=mybir.AluOpType.bypass, op1=mybir.AluOpType.mult)
            v.scalar_tensor_tensor(t5[:], spad_sb[:], 1.0, ohy_sb[:],
                                   accum_out=sy[:], **_stt)
            v.scalar_tensor_tensor(t4[:], haz_sb[:], 1.0, ohy_sb[:, 0:K],
                                   accum_out=hy[:], **_stt)
            v.scalar_tensor_tensor(t5[:], spad_sb[:], 1.0, ohy1_sb[:],
                                   accum_out=sy1[:], **_stt)
            for x in (sy, hy, sy1):
                v.tensor_scalar_max(x[:], x[:], EPS_NLL)
            lsy = small.tile([B, 1], F32)
            lhy = small.tile([B, 1], F32)
            lsy1 = small.tile([B, 1], F32)
            a.activation(lsy[:], sy[:], AF.Ln)
            a.activation(lhy[:], hy[:], AF.Ln)
            a.activation(lsy1[:], sy1[:], AF.Ln)
            tu = small.tile([B, 1], F32)
            tcen = small.tile([B, 1], F32)
            negl = small.tile([B, 1], F32)
            v.tensor_add(tu[:], lsy[:], lhy[:])
            v.tensor_mul(tu[:], tu[:], cfs_sb[:, 1:2])      # *(1-cf)
            v.tensor_mul(tcen[:], lsy1[:], cfs_sb[:, 0:1])  # *cf
            v.tensor_add(negl[:], tu[:], tcen[:])           # = -neg_l per b

            # ---------- intra cohort term ----------
            gp_sb = blob_sb[0:B, OFF_GP:OFF_GP + 512]
            sq_scr = small.tile([B, C], F32)
            ssqi = small.tile([B, 4], F32)
            for p in range(4):
                a.activation(sq_scr[:], ind_sb[:, p * C:(p + 1) * C], AF.Square,
                             accum_out=ssqi[:, p:p + 1])
            rsi = small.tile([B, 4], F32)
            a.activation(rsi[:], ssqi[:], AF.Ln)
            a.activation(rsi[:], rsi[:], AF.Exp, scale=-0.5)
            ssqg = small.tile([B, 2], F32)
            for t in range(2):
                a.activation(sq_scr[:], gp_sb[:, t * C:(t + 1) * C], AF.Square,
                             accum_out=ssqg[:, t:t + 1])
            rsg = small.tile([B, 2], F32)
            a.activation(rsg[:], ssqg[:], AF.Ln)
            a.activation(rsg[:], rsg[:], AF.Exp, scale=-0.5)
            # normalize rows in place (anchor sums already consumed ind_sb)
            for p in range(4):
                v.tensor_scalar_mul(ind_sb[:, p * C:(p + 1) * C],
                                    ind_sb[:, p * C:(p + 1) * C], rsi[:, p:p + 1])
            for t in range(2):
                v.tensor_scalar_mul(gp_sb[:, t * C:(t + 1) * C],
                                    gp_sb[:, t * C:(t + 1) * C], rsg[:, t:t + 1])
            D = small.tile([B, 8], F32)
            prod = small.tile([B, C], F32)
            for p in range(4):
                for t in range(2):
                    col = p * 2 + t
                    v.scalar_tensor_tensor(
                        prod[:], ind_sb[:, p * C:(p + 1) * C], 1.0,
                        gp_sb[:, t * C:(t + 1) * C],
                        op0=mybir.AluOpType.bypass, op1=mybir.AluOpType.mult,
                        accum_out=D[:, col:col + 1])
            U8 = small.tile([B, 8], F32)
            a.activation(U8[:], D[:], AF.Abs)
            # mask==1 entries (cols 0,1,4,7) use -sim instead of |sim|
            v.tensor_scalar_mul(U8[:, 0:2], D[:, 0:2], -1.0)
            v.tensor_scalar_mul(U8[:, 4:5], D[:, 4:5], -1.0)
            v.tensor_scalar_mul(U8[:, 7:8], D[:, 7:8], -1.0)
            isum = small.tile([B, 1], F32)
            v.reduce_sum(isum[:], U8[:], axis=mybir.AxisListType.X)
            # contrib_b = -negl/B + isum/(8B) + 1/B  -> sums to nll + intra_loss
            c1 = small.tile([B, 1], F32)
            c2 = small.tile([B, 1], F32)
            contrib = small.tile([B, 1], F32)
            v.tensor_scalar_mul(c1[:], negl[:], -1.0 / B)
            v.tensor_scalar_mul(c2[:], isum[:], 1.0 / (8 * B))
            v.tensor_add(contrib[:], c1[:], c2[:])
            v.tensor_scalar_add(contrib[:], contrib[:], 1.0 / B)

            # ---------- main loop over bank tiles ----------
            # E_ps[0, k*B + b] accumulates sum_n exp(sims[b, n in class k])
            # across each class's tiles directly in PSUM. The contrib-sum
            # scalar shares the same bank (separate accumulation region).
            E_all = ps_one.tile([1, K * B + 1], F32, tag="E")
            E_ps = E_all[:, 0:K * B].rearrange("p (k b) -> p k b", k=K)
            cs_ps = E_all[:, K * B:K * B + 1]
            # The exp+class-sum of tile t-1 is emitted after tile t's matmuls
            # (one-tile software pipeline) so PE's in-order queue never waits
            # on the ACT round trip before starting the next transposes.
            def _emit_exp_stage(p_all, rh4, k, first, last):
                e_T = epool.tile([128, EPG * B], BF16, name="e_T", tag="e_T")
                for e in range(EPG):
                    a.activation(e_T[:, e * B:(e + 1) * B],
                                 p_all[:, e * B:(e + 1) * B], AF.Exp,
                                 scale=rh4[:, e:e + 1])
                for e in range(EPG):
                    nc.tensor.matmul(E_ps[:, k, :], ones_bf[:],
                                     e_T[:, e * B:(e + 1) * B],
                                     start=(first and e == 0),
                                     stop=(last and e == EPG - 1),
                                     skip_group_check=True)

            pend = None
            for t in range(TILES):
                k = t // TILES_PER_CLASS
                first = (t % TILES_PER_CLASS == 0)
                last = (t % TILES_PER_CLASS == TILES_PER_CLASS - 1)
                if t < 2:
                    T_sb = T_tiles[t]
                else:
                    T_sb = tpool.tile([128, 4096], BF16, name="T_sb", tag="T")
                    # SWDGE cast-DMA: f32 HBM -> bf16 SBUF at line rate
                    nc.gpsimd.dma_start(
                        out=T_sb.rearrange("p (e x) -> p e x", e=EPG),
                        in_=bank[t * NT:(t + 1) * NT, :].rearrange(
                            "(p e) x -> p e x", e=EPG))
                Tv = T_sb.rearrange("p (e j c) -> p e j c", e=EPG, j=4)
                # component sum over j in 2 adds: (j0+j2, j1+j3) then pair-sum
                U_sb = upool.tile([128, 2048], BF16)
                Uv = U_sb.rearrange("p (e g c) -> p e g c", e=EPG, g=2)
                v.tensor_add(Uv[:], Tv[:, :, 0:2, :], Tv[:, :, 2:4, :])
                S_sb = spool.tile([128, 1024], BF16)
                Sv = S_sb.rearrange("p (e c) -> p e c", e=EPG)
                v.tensor_add(Sv[:], Uv[:, :, 0, :], Uv[:, :, 1, :])
                st_ps = [ps_st.tile([128, 512], BF16, name=f"stps{h}", tag=f"stps{h}")
                         for h in range(2)]
                for e in range(EPG):
                    for h in range(2):
                        nc.tensor.transpose(
                            st_ps[h][:, e * 128:(e + 1) * 128],
                            S_sb[:, e * C + h * 128: e * C + (h + 1) * 128],
                            ident_bf[:])
                st_sb = [stpool.tile([128, 512], BF16, name=f"st{h}", tag=f"st{h}")
                         for h in range(2)]
                stsq = [stpool.tile([128, 512], BF16, name=f"sq{h}", tag=f"sq{h}")
                        for h in range(2)]
                for h in range(2):
                    v.tensor_copy(st_sb[h][:], st_ps[h][:])
                    a.activation(stsq[h][:], st_ps[h][:], AF.Square)
                # P_T[n, b] = S_n . A_b with n on partitions: out-partition of
                # block e is the original bank partition p, so rh4[:, e] is a
                # legal per-partition scale for the exp.  ssq rides in the same
                # PSUM bank (cols 256:260), reduced over c by PE ones-matmuls.
                p_all = ps_p.tile([128, EPG * B + EPG], F32)
                for e in range(EPG):
                    for h in range(2):
                        nc.tensor.matmul(p_all[:, e * B:(e + 1) * B],
                                         st_sb[h][:, e * 128:(e + 1) * 128],
                                         at_sb[:, h, :],
                                         start=(h == 0), stop=(h == 1))
                for e in range(EPG):
                    for h in range(2):
                        nc.tensor.matmul(
                            p_all[:, EPG * B + e:EPG * B + e + 1],
                            stsq[h][:, e * 128:(e + 1) * 128], ones_bf[:],
                            start=(h == 0), stop=(h == 1))
                # rh4 = 0.5/||S||  (the ln(1/2) bias folds in the 1/tau factor)
                rh4 = sqpool.tile([128, EPG], F32, tag="rh4")
                a.activation(rh4[:], p_all[:, EPG * B:EPG * B + EPG], AF.Ln)
                a.activation(rh4[:], rh4[:], AF.Exp, scale=-0.5, bias=lnhalf_sb[:])
                if pend is not None:
                    _emit_exp_stage(*pend)
                pend = (p_all, rh4, k, first, last)
            _emit_exp_stage(*pend)

            # ---------- epilogue: partial scalars ----------
            E_row = small.tile([1, K * B], F32)
            a.copy(E_row[:], E_all[:, 0:K * B])
            nc.sync.dma_start(out=e_dbg[:], in_=E_row[:])
            epm = small.tile([1, K * B], F32)
            ep1 = small.tile([1, 1], F32)
            rsum = small.tile([1, 1], F32)
            en1 = small.tile([1, 1], F32)
            v.scalar_tensor_tensor(epm[:], E_row[:], 1.0, oh4t_sb[:],
                                   op0=mybir.AluOpType.bypass,
                                   op1=mybir.AluOpType.mult, accum_out=ep1[:])
            v.reduce_sum(rsum[:], E_row[:], axis=mybir.AxisListType.X)
            v.tensor_scalar_mul(en1[:], ep1[:], -1.0)
            v.tensor_add(en1[:], en1[:], rsum[:])
            # contrib sum over b via PE ones-reduction
            nc.tensor.matmul(cs_ps[:], contrib[:], ones_sb[0:B, :],
                             start=True, stop=True, skip_group_check=True)
            F1 = small.tile([1, 4], F32)
            v.memset(F1[:], 0.0)
            v.tensor_scalar_mul(F1[:, 0:1], ep1[:], 1.0 / (B * NB))
            v.tensor_scalar_mul(F1[:, 1:2], en1[:], 1.0 / (B * (K - 1) * NB))
            v.tensor_copy(F1[:, 2:3], cs_ps[:])
            nc.sync.dma_start(out=out_d[:], in_=F1[:])

    nc.compile()
    return nc


_NC = None


def _get_nc():
    global _NC
    if _NC is None:
        _NC = _build()
    return _NC


def _make_in_maps(hazards, S, indiv, gene, path, cohort_bank, label, c):
    hazards = np.asarray(hazards, dtype=np.float32)
    S = np.asarray(S, dtype=np.float32)
    indiv = np.asarray(indiv, dtype=np.float32)
    gene = np.asarray(gene, dtype=np.float32)
    path = np.asarray(path, dtype=np.float32)
    cohort_bank = np.asarray(cohort_bank, dtype=np.float32)
    label = np.asarray(label)
    c = np.asarray(c)

    blob = np.zeros((128, 1824), np.float32)
    blob[0:B, 0:1024] = indiv.reshape(B, -1)
    blob[0:B, 1024:1536] = np.concatenate(
        [gene.reshape(B, -1), path.reshape(B, -1)], axis=1)
    blob[0:B, 1536:1540] = hazards
    blob[0:B, 1540:1545] = np.concatenate(
        [np.ones((B, 1), np.float32), S], axis=1)
    oh5 = np.zeros((B, K + 1), np.float32)
    oh5[np.arange(B), label] = 1.0
    blob[0:B, 1545:1550] = oh5
    oh5b = np.zeros((B, K + 1), np.float32)
    oh5b[np.arange(B), label + 1] = 1.0
    blob[0:B, 1550:1555] = oh5b
    blob[0:B, 1555] = c.astype(np.float32)
    blob[0:B, 1556] = 1.0 - c.astype(np.float32)
    blob[0, 1557 + label.astype(np.int64) * B + np.arange(B)] = 1.0
    common = dict(blob=blob)
    bankf = cohort_bank.reshape(K, NB, 1024)
    in_maps = []
    for i in range(NCORES):
        shard = np.ascontiguousarray(
            bankf[:, i * NSH:(i + 1) * NSH, :]).reshape(ROWS, 1024)
        in_maps.append({**common, "bank": shard})
    return in_maps


_LAST_RESULTS = None  # stashed for test.py introspection


def kernel(hazards, S, indiv, gene, path, cohort_bank, label, c):
    global _LAST_RESULTS
    nc = _get_nc()
    in_maps = _make_in_maps(hazards, S, indiv, gene, path, cohort_bank, label, c)
    trace = bool(int(os.environ.get("TRNK_TRACE", "0")))
    res = run_bass_kernel_spmd(nc, in_maps, core_ids=list(range(NCORES)),
                               trace=trace)
    _LAST_RESULTS = res
    outs = np.stack([r["out_vec"][0, :] for r in res.results])  # [8, 4]
    ep = float(outs[:, 0].sum())
    en = float(outs[:, 1].sum())
    other = float(outs[:, 2].mean())
    loss = other - math.log((ep + EPS_COH) / (ep + en + EPS_COH))
    return np.float32(loss)


# revision 43
# speedup vs baseline: 1.0178x; 1.0178x over previous
"""Trainium2 Bass kernel for nn_Loss_factory_12429635355015.

Loss = NLLSurv + CohortLoss(intra + inter) over a [4, 8192, 4, 256] cohort bank.

Strategy (memory-bound, 8 NeuronCores):
  - Shard cohort_bank along the N (bank-entry) axis: each core streams its
    16 MiB shard once at HBM line rate (8 tiles x 2 MiB contiguous SWDGE
    cast-DMAs, f32 HBM -> bf16 SBUF).
  - Per 512-entry tile the compute is balanced so every engine stays under
    the ~5.9us/tile DMA floor:
      DVE:  2 adds for the 4-component sum + segmented reduce for ||S||^2.
      PE :  8 transposes, then the anchor matmul in TRANSPOSED orientation:
            P_T[n, b] = st.T @ at, so the out-partition of block e is the
            original bank partition p.
      ACT:  Square; rh4 = exp(-0.5 ln ssq + ln .5) = 0.5/||S||; then
            Exp(P_T, scale=rh4[:, e]) fuses the l2-norm AND 1/tau into the
            exp's per-partition scale. Per-class sums are PE ones-matmuls
            accumulated in PSUM across each class's tiles.
  - NLL + intra terms are computed on-device from host-encoded one-hots
    (index encoding only; all arithmetic on device).
  - Each core outputs [ep_partial, en_partial, nll+intra]; the host sums the
    two scalars across cores (the 'all-reduce two scalars' step) and applies
    the final -log((ep+eps)/(ep+en+eps)).
"""

import math
import os
import sys

import numpy as np

for _p in ("/opt/trn_rl_repo",):
    if _p not in sys.path and os.path.isdir(_p):
        sys.path.insert(0, _p)

import concourse.bacc as bacc
import concourse.tile as tile
from concourse import mybir
from concourse.bass_utils import run_bass_kernel_spmd

# Pin every activation to the one table set that contains all functions this
# kernel uses (Square/Ln/Exp/Copy/Abs/Identity). Without this, Bacc's
# first-match set selection alternates between sets (Ln lives outside the
# default exp set) and reloads the ACT tables ~1.3us per switch every tile.
_ACT_SET = "natural_log_exp_and_others"


def _pin_act_tables():
    import functools
    import concourse.hw_specs as hw_specs
    if getattr(hw_specs.get_activation_tables, "_pinned", False):
        return
    orig = hw_specs.get_activation_tables

    @functools.cache
    def pinned(arch):
        tabs = orig(arch)
        return {k: (v if k == _ACT_SET else set()) for k, v in tabs.items()}

    pinned._pinned = True
    hw_specs.get_activation_tables = pinned
    bacc.get_activation_tables = pinned


_pin_act_tables()

F32 = mybir.dt.float32
AF = mybir.ActivationFunctionType

# Problem constants (hardcoded per spec).
B = 64            # batch
K = 4             # n_cls
C = 256           # feature dim
NB = 8192         # bank entries per class (global)
NCORES = 8
NSH = NB // NCORES          # 1024 bank entries per class per core
ROWS = K * NSH              # 4096 rows of [4*256] per core
NT = 512                    # bank entries per tile (2 MiB)
TILES = ROWS // NT          # 8
TILES_PER_CLASS = NSH // NT # 2
EPG = NT // 128             # 4 entries per partition per tile
LN_HALF = math.log(0.5)
EPS_NLL = 1e-7
EPS_COH = 1e-8


def _build():
    nc = bacc.Bacc("TRN2", target_bir_lowering=False, debug=False,
                   enable_asserts=False, num_devices=NCORES)

    # All small inputs ride in ONE blob DMA so the prologue is never starved
    # behind the saturated bank stream (13 small HWDGE DMAs serialized for
    # ~25us otherwise).  Layout (f32, [128, BLOBW]):
    #   [0:64,    0:1024] indiv      [0:64, 1024:1536] gene|path
    #   [0:64, 1536:1540] hazards    [0:64, 1540:1545] S_padded
    #   [0:64, 1545:1550] onehot(Y)  [0:64, 1550:1555] onehot(Y+1)
    #   [0:64, 1555:1557] [cf, 1-cf] [0:1,  1557:1813] class-of-b mask row
    BLOBW = 1824
    OFF_GP, OFF_HAZ, OFF_SPAD = 1024, 1536, 1540
    OFF_OHY, OFF_OHY1, OFF_CFS, OFF_OH4T = 1545, 1550, 1555, 1557
    bank = nc.dram_tensor("bank", [ROWS, 1024], F32, kind="ExternalInput")
    blob = nc.dram_tensor("blob", [128, BLOBW], F32, kind="ExternalInput")

    out_d = nc.dram_tensor("out_vec", [1, 4], F32, kind="ExternalOutput")
    e_dbg = nc.dram_tensor("e_dbg", [1, K * B], F32, kind="ExternalOutput")

    import ml_dtypes
    cst_f = np.zeros((128, 129), dtype=np.float32)
    cst_f[:, 0:128] = np.eye(128, dtype=np.float32)
    cst_f[:, 128] = 1.0
    cst_b = np.zeros((128, 129), dtype=ml_dtypes.bfloat16)
    cst_b[:, 0:128] = np.eye(128, dtype=ml_dtypes.bfloat16)
    cst_b[:, 128] = 1.0
    cst_f_d = nc.inline_tensor(cst_f, "cst_f")
    cst_b_d = nc.inline_tensor(cst_b, "cst_b")

    v = nc.vector
    a = nc.scalar

    with tile.TileContext(nc) as tc:
        from contextlib import ExitStack
        with ExitStack() as ctx:
            const = ctx.enter_context(tc.tile_pool(name="const", bufs=1))
            small = ctx.enter_context(tc.tile_pool(name="small", bufs=1))
            tpool = ctx.enter_context(tc.tile_pool(name="T", bufs=6))
            upool = ctx.enter_context(tc.tile_pool(name="U", bufs=2))
            spool = ctx.enter_context(tc.tile_pool(name="S", bufs=2))
            sqpool = ctx.enter_context(tc.tile_pool(name="sq", bufs=2))
            stpool = ctx.enter_context(tc.tile_pool(name="STsb", bufs=2))
            epool = ctx.enter_context(tc.tile_pool(name="esb", bufs=2))
            ps_st = ctx.enter_context(tc.tile_pool(name="ps_st", bufs=2, space="PSUM"))
            ps_p = ctx.enter_context(tc.tile_pool(name="ps_p", bufs=2, space="PSUM"))
            ps_one = ctx.enter_context(tc.tile_pool(name="ps_one", bufs=1, space="PSUM"))

            BF16 = mybir.dt.bfloat16
            # Bank tiles 0/1 go first on the SWDGE queue so the big stream
            # starts immediately; the blob rides behind them (it is only
            # needed once the anchors are computed, ~15us in).
            T_tiles = []
            for t in range(2):
                T_sb = tpool.tile([128, 4096], BF16, name="T_sb", tag="T")
                nc.gpsimd.dma_start(
                    out=T_sb.rearrange("p (e x) -> p e x", e=EPG),
                    in_=bank[t * NT:(t + 1) * NT, :].rearrange(
                        "(p e) x -> p e x", e=EPG))
                T_tiles.append(T_sb)
            blob_sb = small.tile([128, BLOBW], F32)
            nc.gpsimd.dma_start(out=blob_sb[:], in_=blob[:])
            cst_f_sb = const.tile([128, 129], F32)
            nc.sync.dma_start(out=cst_f_sb[:], in_=cst_f_d[:])
            cst_b_sb = const.tile([128, 129], BF16)
            nc.sync.dma_start(out=cst_b_sb[:], in_=cst_b_d[:])
            ident_sb = cst_f_sb[:, 0:128]
            ones_sb = cst_f_sb[:, 128:129]
            ident_bf = cst_b_sb[:, 0:128]
            ones_bf = cst_b_sb[:, 128:129]
            lnhalf_sb = const.tile([128, 1], F32)
            v.memset(lnhalf_sb[:], LN_HALF)

            # ---------- anchors: A = l2norm(mean_j indiv[b,j,:]) ----------
            ind_sb = blob_sb[0:B, 0:1024]
            iv = ind_sb.rearrange("p (j c) -> p j c", j=4)
            asum = small.tile([B, C], F32)
            atmp = small.tile([B, C], F32)
            v.tensor_add(asum[:], iv[:, 0, :], iv[:, 1, :])
            v.tensor_add(atmp[:], iv[:, 2, :], iv[:, 3, :])
            v.tensor_add(asum[:], asum[:], atmp[:])
            sqa = small.tile([B, C], F32)
            ssa = small.tile([B, 1], F32)
            a.activation(sqa[:], asum[:], AF.Square, accum_out=ssa[:])
            lna = small.tile([B, 1], F32)
            a.activation(lna[:], ssa[:], AF.Ln)
            rsa = small.tile([B, 1], F32)
            a.activation(rsa[:], lna[:], AF.Exp, scale=-0.5)
            v.tensor_scalar_mul(asum[:], asum[:], rsa[:])
            at_ps = ps_one.tile([128, 2, B], F32, tag="at")
            for h in range(2):
                nc.tensor.transpose(at_ps[:, h, :], asum[:, h * 128:(h + 1) * 128],
                                    ident_sb[0:B, 0:B])
            at_sb = const.tile([128, 2, B], BF16)
            a.copy(at_sb[:], at_ps[:])

            # ---------- NLL (per-b, b on partitions) ----------
            haz_sb = blob_sb[0:B, OFF_HAZ:OFF_HAZ + K]
            spad_sb = blob_sb[0:B, OFF_SPAD:OFF_SPAD + K + 1]
            ohy_sb = blob_sb[0:B, OFF_OHY:OFF_OHY + K + 1]
            ohy1_sb = blob_sb[0:B, OFF_OHY1:OFF_OHY1 + K + 1]
            oh4t_sb = blob_sb[0:1, OFF_OH4T:OFF_OH4T + K * B]
            cfs_sb = blob_sb[0:B, OFF_CFS:OFF_CFS + 2]

            t5 = small.tile([B, K + 1], F32)
            t4 = small.tile([B, K], F32)
            sy = small.tile([B, 1], F32)
            hy = small.tile([B, 1], F32)
            sy1 = small.tile([B, 1], F32)
            _stt = dict(op0=mybir.AluOpType.bypass, op1=mybir.AluOpType.mult)
            v.scalar_tensor_tensor(t5[:], spad_sb[:], 1.0, ohy_sb[:],
                                   accum_out=sy[:], **_stt)
            v.scalar_tensor_tensor(t4[:], haz_sb[:], 1.0, ohy_sb[:, 0:K],
                                   accum_out=hy[:], **_stt)
            v.scalar_tensor_tensor(t5[:], spad_sb[:], 1.0, ohy1_sb[:],
                                   accum_out=sy1[:], **_stt)
            for x in (sy, hy, sy1):
                v.tensor_scalar_max(x[:], x[:], EPS_NLL)
            lsy = small.tile([B, 1], F32)
            lhy = small.tile([B, 1], F32)
            lsy1 = small.tile([B, 1], F32)
            a.activation(lsy[:], sy[:], AF.Ln)
            a.activation(lhy[:], hy[:], AF.Ln)
            a.activation(lsy1[:], sy1[:], AF.Ln)
            tu = small.tile([B, 1], F32)
            tcen = small.tile([B, 1], F32)
            negl = small.tile([B, 1], F32)
            v.tensor_add(tu[:], lsy[:], lhy[:])
            v.tensor_mul(tu[:], tu[:], cfs_sb[:, 1:2])      # *(1-cf)
            v.tensor_mul(tcen[:], lsy1[:], cfs_sb[:, 0:1])  # *cf
            v.tensor_add(negl[:], tu[:], tcen[:])           # = -neg_l per b

            # ---------- intra cohort term ----------
            gp_sb = blob_sb[0:B, OFF_GP:OFF_GP + 512]
            sq_scr = small.tile([B, C], F32)
            ssqi = small.tile([B, 4], F32)
            for p in range(4):
                a.activation(sq_scr[:], ind_sb[:, p * C:(p + 1) * C], AF.Square,
                             accum_out=ssqi[:, p:p + 1])
            rsi = small.tile([B, 4], F32)
            a.activation(rsi[:], ssqi[:], AF.Ln)
            a.activation(rsi[:], rsi[:], AF.Exp, scale=-0.5)
            ssqg = small.tile([B, 2], F32)
            for t in range(2):
                a.activation(sq_scr[:], gp_sb[:, t * C:(t + 1) * C], AF.Square,
                             accum_out=ssqg[:, t:t + 1])
            rsg = small.tile([B, 2], F32)
            a.activation(rsg[:], ssqg[:], AF.Ln)
            a.activation(rsg[:], rsg[:], AF.Exp, scale=-0.5)
            # normalize rows in place (anchor sums already consumed ind_sb)
            for p in range(4):
                v.tensor_scalar_mul(ind_sb[:, p * C:(p + 1) * C],
                                    ind_sb[:, p * C:(p + 1) * C], rsi[:, p:p + 1])
            for t in range(2):
                v.tensor_scalar_mul(gp_sb[:, t * C:(t + 1) * C],
                                    gp_sb[:, t * C:(t + 1) * C], rsg[:, t:t + 1])
            D = small.tile([B, 8], F32)
            prod = small.tile([B, C], F32)
            for p in range(4):
                for t in range(2):
                    col = p * 2 + t
                    v.scalar_tensor_tensor(
                        prod[:], ind_sb[:, p * C:(p + 1) * C], 1.0,
                        gp_sb[:, t * C:(t + 1) * C],
                        op0=mybir.AluOpType.bypass, op1=mybir.AluOpType.mult,
                        accum_out=D[:, col:col + 1])
            U8 = small.tile([B, 8], F32)
            a.activation(U8[:], D[:], AF.Abs)
            # mask==1 entries (cols 0,1,4,7) use -sim instead of |sim|
            v.tensor_scalar_mul(U8[:, 0:2], D[:, 0:2], -1.0)
            v.tensor_scalar_mul(U8[:, 4:5], D[:, 4:5], -1.0)
            v.tensor_scalar_mul(U8[:, 7:8], D[:, 7:8], -1.0)
            isum = small.tile([B, 1], F32)
            v.reduce_sum(isum[:], U8[:], axis=mybir.AxisListType.X)
            # contrib_b = -negl/B + isum/(8B) + 1/B  -> sums to nll + intra_loss
            c1 = small.tile([B, 1], F32)
            c2 = small.tile([B, 1], F32)
            contrib = small.tile([B, 1], F32)
            v.tensor_scalar_mul(c1[:], negl[:], -1.0 / B)
            v.tensor_scalar_mul(c2[:], isum[:], 1.0 / (8 * B))
            v.tensor_add(contrib[:], c1[:], c2[:])
            v.tensor_scalar_add(contrib[:], contrib[:], 1.0 / B)

            # ---------- main loop over bank tiles ----------
            # E_ps[0, k*B + b] accumulates sum_n exp(sims[b, n in class k])
            # across each class's tiles directly in PSUM. The contrib-sum
            # scalar shares the same bank (separate accumulation region).
            E_all = ps_one.tile([1, K * B + 1], F32, tag="E")
            E_ps = E_all[:, 0:K * B].rearrange("p (k b) -> p k b", k=K)
            cs_ps = E_all[:, K * B:K * B + 1]
            # The exp+class-sum of tile t-1 is emitted after tile t's matmuls
            # (one-tile software pipeline) so PE's in-order queue never waits
            # on the ACT round trip before starting the next transposes.
            def _emit_exp_stage(p_all, rh4, k, first, last):
                e_T = epool.tile([128, EPG * B], BF16, name="e_T", tag="e_T")
                for e in range(EPG):
                    a.activation(e_T[:, e * B:(e + 1) * B],
                                 p_all[:, e * B:(e + 1) * B], AF.Exp,
                                 scale=rh4[:, e:e + 1])
                for e in range(EPG):
                    nc.tensor.matmul(E_ps[:, k, :], ones_bf[:],
                                     e_T[:, e * B:(e + 1) * B],
                                     start=(first and e == 0),
                                     stop=(last and e == EPG - 1),
                                     skip_group_check=True)

            pend = None
            for t in range(TILES):
                k = t // TILES_PER_CLASS
                first = (t % TILES_PER_CLASS == 0)
                last = (t % TILES_PER_CLASS == TILES_PER_CLASS - 1)
                if t < 2:
                    T_sb = T_tiles[t]
                else:
                    T_sb = tpool.tile([128, 4096], BF16, name="T_sb", tag="T")
                    # SWDGE cast-DMA: f32 HBM -> bf16 SBUF at line rate
                    nc.gpsimd.dma_start(
                        out=T_sb.rearrange("p (e x) -> p e x", e=EPG),
                        in_=bank[t * NT:(t + 1) * NT, :].rearrange(
                            "(p e) x -> p e x", e=EPG))
                Tv = T_sb.rearrange("p (e j c) -> p e j c", e=EPG, j=4)
                # component sum over j in 2 adds: (j0+j2, j1+j3) then pair-sum
                U_sb = upool.tile([128, 2048], BF16)
                Uv = U_sb.rearrange("p (e g c) -> p e g c", e=EPG, g=2)
                v.tensor_add(Uv[:], Tv[:, :, 0:2, :], Tv[:, :, 2:4, :])
                S_sb = spool.tile([128, 1024], BF16)
                Sv = S_sb.rearrange("p (e c) -> p e c", e=EPG)
                v.tensor_add(Sv[:], Uv[:, :, 0, :], Uv[:, :, 1, :])
                st_ps = [ps_st.tile([128, 512], BF16, name=f"stps{h}", tag=f"stps{h}")
                         for h in range(2)]
                for e in range(EPG):
                    for h in range(2):
                        nc.tensor.transpose(
                            st_ps[h][:, e * 128:(e + 1) * 128],
                            S_sb[:, e * C + h * 128: e * C + (h + 1) * 128],
                            ident_bf[:])
                st_sb = [stpool.tile([128, 512], BF16, name=f"st{h}", tag=f"st{h}")
                         for h in range(2)]
                stsq = [stpool.tile([128, 512], BF16, name=f"sq{h}", tag=f"sq{h}")
                        for h in range(2)]
                for h in range(2):
                    v.tensor_copy(st_sb[h][:], st_ps[h][:])
                    a.activation(stsq[h][:], st_ps[h][:], AF.Square)
                # P_T[n, b] = S_n . A_b with n on partitions: out-partition of
                # block e is the original bank partition p, so rh4[:, e] is a
                # legal per-partition scale for the exp.  ssq rides in the same
                # PSUM bank (cols 256:260), reduced over c by PE ones-matmuls.
                p_all = ps_p.tile([128, EPG * B + EPG], F32)
                for e in range(EPG):
                    for h in range(2):
                        nc.tensor.matmul(p_all[:, e * B:(e + 1) * B],
                                         st_sb[h][:, e * 128:(e + 1) * 128],
                                         at_sb[:, h, :],
                                         start=(h == 0), stop=(h == 1))
                for e in range(EPG):
                    for h in range(2):
                        nc.tensor.matmul(
                            p_all[:, EPG * B + e:EPG * B + e + 1],
                            stsq[h][:, e * 128:(e + 1) * 128], ones_bf[:],
                            start=(h == 0), stop=(h == 1))
                # rh4 = 0.5/||S||  (the ln(1/2) bias folds in the 1/tau factor)
                rh4 = sqpool.tile([128, EPG], F32, tag="rh4")
                a.activation(rh4[:], p_all[:, EPG * B:EPG * B + EPG], AF.Ln)
                a.activation(rh4[:], rh4[:], AF.Exp, scale=-0.5, bias=lnhalf_sb[:])
                if pend is not None:
                    _emit_exp_stage(*pend)
                pend = (p_all, rh4, k, first, last)
            _emit_exp_stage(*pend)

            # ---------- epilogue: partial scalars ----------
            E_row = small.tile([1, K * B], F32)
            a.copy(E_row[:], E_all[:, 0:K * B])
            nc.sync.dma_start(out=e_dbg[:], in_=E_row[:])
            epm = small.tile([1, K * B], F32)
            ep1 = small.tile([1, 1], F32)
            rsum = small.tile([1, 1], F32)
            en1 = small.tile([1, 1], F32)
            v.scalar_tensor_tensor(epm[:], E_row[:], 1.0, oh4t_sb[:],
                                   op0=mybir.AluOpType.bypass,
                                   op1=mybir.AluOpType.mult, accum_out=ep1[:])
            v.reduce_sum(rsum[:], E_row[:], axis=mybir.AxisListType.X)
            v.tensor_scalar_mul(en1[:], ep1[:], -1.0)
            v.tensor_add(en1[:], en1[:], rsum[:])
            # contrib sum over b via PE ones-reduction
            nc.tensor.matmul(cs_ps[:], contrib[:], ones_sb[0:B, :],
                             start=True, stop=True, skip_group_check=True)
            F1 = small.tile([1, 4], F32)
            v.memset(F1[:], 0.0)
            v.tensor_scalar_mul(F1[:, 0:1], ep1[:], 1.0 / (B * NB))
            v.tensor_scalar_mul(F1[:, 1:2], en1[:], 1.0 / (B * (K - 1) * NB))
            v.tensor_copy(F1[:, 2:3], cs_ps[:])
            nc.sync.dma_start(out=out_d[:], in_=F1[:])

    nc.compile()
    return nc


_NC = None


def _get_nc():
    global _NC
    if _NC is None:
        _NC = _build()
    return _NC


def _make_in_maps(hazards, S, indiv, gene, path, cohort_bank, label, c):
    hazards = np.asarray(hazards, dtype=np.float32)
    S = np.asarray(S, dtype=np.float32)
    indiv = np.asarray(indiv, dtype=np.float32)
    gene = np.asarray(gene, dtype=np.float32)
    path = np.asarray(path, dtype=np.float32)
    cohort_bank = np.asarray(cohort_bank, dtype=np.float32)
    label = np.asarray(label)
    c = np.asarray(c)

    blob = np.zeros((128, 1824), np.float32)
    blob[0:B, 0:1024] = indiv.reshape(B, -1)
    blob[0:B, 1024:1536] = np.concatenate(
        [gene.reshape(B, -1), path.reshape(B, -1)], axis=1)
    blob[0:B, 1536:1540] = hazards
    blob[0:B, 1540:1545] = np.concatenate(
        [np.ones((B, 1), np.float32), S], axis=1)
    oh5 = np.zeros((B, K + 1), np.float32)
    oh5[np.arange(B), label] = 1.0
    blob[0:B, 1545:1550] = oh5
    oh5b = np.zeros((B, K + 1), np.float32)
    oh5b[np.arange(B), label + 1] = 1.0
    blob[0:B, 1550:1555] = oh5b
    blob[0:B, 1555] = c.astype(np.float32)
    blob[0:B, 1556] = 1.0 - c.astype(np.float32)
    blob[0, 1557 + label.astype(np.int64) * B + np.arange(B)] = 1.0
    common = dict(blob=blob)
    bankf = cohort_bank.reshape(K, NB, 1024)
    in_maps = []
    for i in range(NCORES):
        shard = np.ascontiguousarray(
            bankf[:, i * NSH:(i + 1) * NSH, :]).reshape(ROWS, 1024)
        in_maps.append({**common, "bank": shard})
    return in_maps


_LAST_RESULTS = None  # stashed for test.py introspection


def kernel(hazards, S, indiv, gene, path, cohort_bank, label, c):
    global _LAST_RESULTS
    nc = _get_nc()
    in_maps = _make_in_maps(hazards, S, indiv, gene, path, cohort_bank, label, c)
    trace = bool(int(os.environ.get("TRNK_TRACE", "0")))
    res = run_bass_kernel_spmd(nc, in_maps, core_ids=list(range(NCORES)),
                               trace=trace)
    _LAST_RESULTS = res
    outs = np.stack([r["out_vec"][0, :] for r in res.results])  # [8, 4]
    ep = float(outs[:, 0].sum())
    en = float(outs[:, 1].sum())
    other = float(outs[:, 2].mean())
    loss = other - math.log((ep + EPS_COH) / (ep + en + EPS_COH))
    return np.float32(loss)


# revision 44
# speedup vs baseline: 1.0223x; 1.0045x over previous
"""Trainium2 Bass kernel for nn_Loss_factory_12429635355015.

Loss = NLLSurv + CohortLoss(intra + inter) over a [4, 8192, 4, 256] cohort bank.

Strategy (memory-bound, 8 NeuronCores):
  - Shard cohort_bank along the N (bank-entry) axis: each core streams its
    16 MiB shard once at HBM line rate (8 tiles x 2 MiB contiguous SWDGE
    cast-DMAs, f32 HBM -> bf16 SBUF).
  - Per 512-entry tile the compute is balanced so every engine stays under
    the ~5.9us/tile DMA floor:
      DVE:  2 adds for the 4-component sum + segmented reduce for ||S||^2.
      PE :  8 transposes, then the anchor matmul in TRANSPOSED orientation:
            P_T[n, b] = st.T @ at, so the out-partition of block e is the
            original bank partition p.
      ACT:  Square; rh4 = exp(-0.5 ln ssq + ln .5) = 0.5/||S||; then
            Exp(P_T, scale=rh4[:, e]) fuses the l2-norm AND 1/tau into the
            exp's per-partition scale. Per-class sums are PE ones-matmuls
            accumulated in PSUM across each class's tiles.
  - NLL + intra terms are computed on-device from host-encoded one-hots
    (index encoding only; all arithmetic on device).
  - Each core outputs [ep_partial, en_partial, nll+intra]; the host sums the
    two scalars across cores (the 'all-reduce two scalars' step) and applies
    the final -log((ep+eps)/(ep+en+eps)).
"""

import math
import os
import sys

import numpy as np

for _p in ("/opt/trn_rl_repo",):
    if _p not in sys.path and os.path.isdir(_p):
        sys.path.insert(0, _p)

import concourse.bacc as bacc
import concourse.tile as tile
from concourse import mybir
from concourse.bass_utils import run_bass_kernel_spmd

# Pin every activation to the one table set that contains all functions this
# kernel uses (Square/Ln/Exp/Copy/Abs/Identity). Without this, Bacc's
# first-match set selection alternates between sets (Ln lives outside the
# default exp set) and reloads the ACT tables ~1.3us per switch every tile.
_ACT_SET = "natural_log_exp_and_others"


def _pin_act_tables():
    import functools
    import concourse.hw_specs as hw_specs
    if getattr(hw_specs.get_activation_tables, "_pinned", False):
        return
    orig = hw_specs.get_activation_tables

    @functools.cache
    def pinned(arch):
        tabs = orig(arch)
        return {k: (v if k == _ACT_SET else set()) for k, v in tabs.items()}

    pinned._pinned = True
    hw_specs.get_activation_tables = pinned
    bacc.get_activation_tables = pinned


_pin_act_tables()

F32 = mybir.dt.float32
AF = mybir.ActivationFunctionType

# Problem constants (hardcoded per spec).
B = 64            # batch
K = 4             # n_cls
C = 256           # feature dim
NB = 8192         # bank entries per class (global)
NCORES = 8
NSH = NB // NCORES          # 1024 bank entries per class per core
ROWS = K * NSH              # 4096 rows of [4*256] per core
NT = 512                    # bank entries per tile (2 MiB)
TILES = ROWS // NT          # 8
TILES_PER_CLASS = NSH // NT # 2
EPG = NT // 128             # 4 entries per partition per tile
LN_HALF = math.log(0.5)
EPS_NLL = 1e-7
EPS_COH = 1e-8


def _build():
    nc = bacc.Bacc("TRN2", target_bir_lowering=False, debug=False,
                   enable_asserts=False, num_devices=NCORES)

    # All small inputs ride in ONE blob DMA so the prologue is never starved
    # behind the saturated bank stream (13 small HWDGE DMAs serialized for
    # ~25us otherwise).  Layout (f32, [128, BLOBW]):
    #   [0:64,    0:1024] indiv      [0:64, 1024:1536] gene|path
    #   [0:64, 1536:1540] hazards    [0:64, 1540:1545] S_padded
    #   [0:64, 1545:1550] onehot(Y)  [0:64, 1550:1555] onehot(Y+1)
    #   [0:64, 1555:1557] [cf, 1-cf] [0:1,  1557:1813] class-of-b mask row
    BLOBW = 1824
    OFF_GP, OFF_HAZ, OFF_SPAD = 1024, 1536, 1540
    OFF_OHY, OFF_OHY1, OFF_CFS, OFF_OH4T = 1545, 1550, 1555, 1557
    bank = nc.dram_tensor("bank", [ROWS, 1024], F32, kind="ExternalInput")
    blob = nc.dram_tensor("blob", [128, BLOBW], F32, kind="ExternalInput")

    out_d = nc.dram_tensor("out_vec", [1, 4], F32, kind="ExternalOutput")
    e_dbg = nc.dram_tensor("e_dbg", [1, K * B], F32, kind="ExternalOutput")

    import ml_dtypes
    cst_f = np.zeros((128, 129), dtype=np.float32)
    cst_f[:, 0:128] = np.eye(128, dtype=np.float32)
    cst_f[:, 128] = 1.0
    cst_b = np.zeros((128, 129), dtype=ml_dtypes.bfloat16)
    cst_b[:, 0:128] = np.eye(128, dtype=ml_dtypes.bfloat16)
    cst_b[:, 128] = 1.0
    cst_f_d = nc.inline_tensor(cst_f, "cst_f")
    cst_b_d = nc.inline_tensor(cst_b, "cst_b")

    v = nc.vector
    a = nc.scalar

    with tile.TileContext(nc) as tc:
        from contextlib import ExitStack
        with ExitStack() as ctx:
            const = ctx.enter_context(tc.tile_pool(name="const", bufs=1))
            small = ctx.enter_context(tc.tile_pool(name="small", bufs=1))
            tpool = ctx.enter_context(tc.tile_pool(name="T", bufs=4))
            upool = ctx.enter_context(tc.tile_pool(name="U", bufs=2))
            spool = ctx.enter_context(tc.tile_pool(name="S", bufs=2))
            sqpool = ctx.enter_context(tc.tile_pool(name="sq", bufs=2))
            stpool = ctx.enter_context(tc.tile_pool(name="STsb", bufs=2))
            epool = ctx.enter_context(tc.tile_pool(name="esb", bufs=2))
            ps_st = ctx.enter_context(tc.tile_pool(name="ps_st", bufs=2, space="PSUM"))
            ps_p = ctx.enter_context(tc.tile_pool(name="ps_p", bufs=2, space="PSUM"))
            ps_one = ctx.enter_context(tc.tile_pool(name="ps_one", bufs=1, space="PSUM"))

            BF16 = mybir.dt.bfloat16
            # Bank tiles 0/1 go first on the SWDGE queue so the big stream
            # starts immediately; the blob rides behind them (it is only
            # needed once the anchors are computed, ~15us in).
            T_tiles = []
            for t in range(2):
                T_sb = tpool.tile([128, 4096], BF16, name="T_sb", tag="T")
                nc.gpsimd.dma_start(
                    out=T_sb.rearrange("p (e x) -> p e x", e=EPG),
                    in_=bank[t * NT:(t + 1) * NT, :].rearrange(
                        "(p e) x -> p e x", e=EPG))
                T_tiles.append(T_sb)
            blob_sb = small.tile([128, BLOBW], F32)
            nc.gpsimd.dma_start(out=blob_sb[:], in_=blob[:])
            cst_f_sb = const.tile([128, 129], F32)
            nc.sync.dma_start(out=cst_f_sb[:], in_=cst_f_d[:])
            cst_b_sb = const.tile([128, 129], BF16)
            nc.sync.dma_start(out=cst_b_sb[:], in_=cst_b_d[:])
            ident_sb = cst_f_sb[:, 0:128]
            ones_sb = cst_f_sb[:, 128:129]
            ident_bf = cst_b_sb[:, 0:128]
            ones_bf = cst_b_sb[:, 128:129]
            lnhalf_sb = const.tile([128, 1], F32)
            v.memset(lnhalf_sb[:], LN_HALF)

            # ---------- anchors: A = l2norm(mean_j indiv[b,j,:]) ----------
            ind_sb = blob_sb[0:B, 0:1024]
            iv = ind_sb.rearrange("p (j c) -> p j c", j=4)
            asum = small.tile([B, C], F32)
            atmp = small.tile([B, C], F32)
            v.tensor_add(asum[:], iv[:, 0, :], iv[:, 1, :])
            v.tensor_add(atmp[:], iv[:, 2, :], iv[:, 3, :])
            v.tensor_add(asum[:], asum[:], atmp[:])
            sqa = small.tile([B, C], F32)
            ssa = small.tile([B, 1], F32)
            a.activation(sqa[:], asum[:], AF.Square, accum_out=ssa[:])
            lna = small.tile([B, 1], F32)
            a.activation(lna[:], ssa[:], AF.Ln)
            rsa = small.tile([B, 1], F32)
            a.activation(rsa[:], lna[:], AF.Exp, scale=-0.5)
            v.tensor_scalar_mul(asum[:], asum[:], rsa[:])
            at_ps = ps_one.tile([128, 2, B], F32, tag="at")
            for h in range(2):
                nc.tensor.transpose(at_ps[:, h, :], asum[:, h * 128:(h + 1) * 128],
                                    ident_sb[0:B, 0:B])
            at_sb = const.tile([128, 2, B], BF16)
            a.copy(at_sb[:], at_ps[:])

            # ---------- NLL (per-b, b on partitions) ----------
            haz_sb = blob_sb[0:B, OFF_HAZ:OFF_HAZ + K]
            spad_sb = blob_sb[0:B, OFF_SPAD:OFF_SPAD + K + 1]
            ohy_sb = blob_sb[0:B, OFF_OHY:OFF_OHY + K + 1]
            ohy1_sb = blob_sb[0:B, OFF_OHY1:OFF_OHY1 + K + 1]
            oh4t_sb = blob_sb[0:1, OFF_OH4T:OFF_OH4T + K * B]
            cfs_sb = blob_sb[0:B, OFF_CFS:OFF_CFS + 2]

            t5 = small.tile([B, K + 1], F32)
            t4 = small.tile([B, K], F32)
            sy = small.tile([B, 1], F32)
            hy = small.tile([B, 1], F32)
            sy1 = small.tile([B, 1], F32)
            _stt = dict(op0=mybir.AluOpType.bypass, op1=mybir.AluOpType.mult)
            v.scalar_tensor_tensor(t5[:], spad_sb[:], 1.0, ohy_sb[:],
                                   accum_out=sy[:], **_stt)
            v.scalar_tensor_tensor(t4[:], haz_sb[:], 1.0, ohy_sb[:, 0:K],
                                   accum_out=hy[:], **_stt)
            v.scalar_tensor_tensor(t5[:], spad_sb[:], 1.0, ohy1_sb[:],
                                   accum_out=sy1[:], **_stt)
            for x in (sy, hy, sy1):
                v.tensor_scalar_max(x[:], x[:], EPS_NLL)
            lsy = small.tile([B, 1], F32)
            lhy = small.tile([B, 1], F32)
            lsy1 = small.tile([B, 1], F32)
            a.activation(lsy[:], sy[:], AF.Ln)
            a.activation(lhy[:], hy[:], AF.Ln)
            a.activation(lsy1[:], sy1[:], AF.Ln)
            tu = small.tile([B, 1], F32)
            tcen = small.tile([B, 1], F32)
            negl = small.tile([B, 1], F32)
            v.tensor_add(tu[:], lsy[:], lhy[:])
            v.tensor_mul(tu[:], tu[:], cfs_sb[:, 1:2])      # *(1-cf)
            v.tensor_mul(tcen[:], lsy1[:], cfs_sb[:, 0:1])  # *cf
            v.tensor_add(negl[:], tu[:], tcen[:])           # = -neg_l per b

            # ---------- intra cohort term ----------
            gp_sb = blob_sb[0:B, OFF_GP:OFF_GP + 512]
            sq_scr = small.tile([B, C], F32)
            ssqi = small.tile([B, 4], F32)
            for p in range(4):
                a.activation(sq_scr[:], ind_sb[:, p * C:(p + 1) * C], AF.Square,
                             accum_out=ssqi[:, p:p + 1])
            rsi = small.tile([B, 4], F32)
            a.activation(rsi[:], ssqi[:], AF.Ln)
            a.activation(rsi[:], rsi[:], AF.Exp, scale=-0.5)
            ssqg = small.tile([B, 2], F32)
            for t in range(2):
                a.activation(sq_scr[:], gp_sb[:, t * C:(t + 1) * C], AF.Square,
                             accum_out=ssqg[:, t:t + 1])
            rsg = small.tile([B, 2], F32)
            a.activation(rsg[:], ssqg[:], AF.Ln)
            a.activation(rsg[:], rsg[:], AF.Exp, scale=-0.5)
            # normalize rows in place (anchor sums already consumed ind_sb)
            for p in range(4):
                v.tensor_scalar_mul(ind_sb[:, p * C:(p + 1) * C],
                                    ind_sb[:, p * C:(p + 1) * C], rsi[:, p:p + 1])
            for t in range(2):
                v.tensor_scalar_mul(gp_sb[:, t * C:(t + 1) * C],
                                    gp_sb[:, t * C:(t + 1) * C], rsg[:, t:t + 1])
            D = small.tile([B, 8], F32)
            prod = small.tile([B, C], F32)
            for p in range(4):
                for t in range(2):
                    col = p * 2 + t
                    v.scalar_tensor_tensor(
                        prod[:], ind_sb[:, p * C:(p + 1) * C], 1.0,
                        gp_sb[:, t * C:(t + 1) * C],
                        op0=mybir.AluOpType.bypass, op1=mybir.AluOpType.mult,
                        accum_out=D[:, col:col + 1])
            U8 = small.tile([B, 8], F32)
            a.activation(U8[:], D[:], AF.Abs)
            # mask==1 entries (cols 0,1,4,7) use -sim instead of |sim|
            v.tensor_scalar_mul(U8[:, 0:2], D[:, 0:2], -1.0)
            v.tensor_scalar_mul(U8[:, 4:5], D[:, 4:5], -1.0)
            v.tensor_scalar_mul(U8[:, 7:8], D[:, 7:8], -1.0)
            isum = small.tile([B, 1], F32)
            v.reduce_sum(isum[:], U8[:], axis=mybir.AxisListType.X)
            # contrib_b = -negl/B + isum/(8B) + 1/B  -> sums to nll + intra_loss
            c1 = small.tile([B, 1], F32)
            c2 = small.tile([B, 1], F32)
            contrib = small.tile([B, 1], F32)
            v.tensor_scalar_mul(c1[:], negl[:], -1.0 / B)
            v.tensor_scalar_mul(c2[:], isum[:], 1.0 / (8 * B))
            v.tensor_add(contrib[:], c1[:], c2[:])
            v.tensor_scalar_add(contrib[:], contrib[:], 1.0 / B)

            # ---------- main loop over bank tiles ----------
            # E_ps[0, k*B + b] accumulates sum_n exp(sims[b, n in class k])
            # across each class's tiles directly in PSUM. The contrib-sum
            # scalar shares the same bank (separate accumulation region).
            E_all = ps_one.tile([1, K * B + 1], F32, tag="E")
            E_ps = E_all[:, 0:K * B].rearrange("p (k b) -> p k b", k=K)
            cs_ps = E_all[:, K * B:K * B + 1]
            # The exp+class-sum of tile t-1 is emitted after tile t's matmuls
            # (one-tile software pipeline) so PE's in-order queue never waits
            # on the ACT round trip before starting the next transposes.
            def _emit_exp_stage(p_all, rh4, k, first, last):
                e_T = epool.tile([128, EPG * B], BF16, name="e_T", tag="e_T")
                for e in range(EPG):
                    a.activation(e_T[:, e * B:(e + 1) * B],
                                 p_all[:, e * B:(e + 1) * B], AF.Exp,
                                 scale=rh4[:, e:e + 1])
                for e in range(EPG):
                    nc.tensor.matmul(E_ps[:, k, :], ones_bf[:],
                                     e_T[:, e * B:(e + 1) * B],
                                     start=(first and e == 0),
                                     stop=(last and e == EPG - 1),
                                     skip_group_check=True)

            pend = None
            for t in range(TILES):
                k = t // TILES_PER_CLASS
                first = (t % TILES_PER_CLASS == 0)
                last = (t % TILES_PER_CLASS == TILES_PER_CLASS - 1)
                if t < 2:
                    T_sb = T_tiles[t]
                else:
                    T_sb = tpool.tile([128, 4096], BF16, name="T_sb", tag="T")
                    # SWDGE cast-DMA: f32 HBM -> bf16 SBUF at line rate
                    nc.gpsimd.dma_start(
                        out=T_sb.rearrange("p (e x) -> p e x", e=EPG),
                        in_=bank[t * NT:(t + 1) * NT, :].rearrange(
                            "(p e) x -> p e x", e=EPG))
                Tv = T_sb.rearrange("p (e j c) -> p e j c", e=EPG, j=4)
                # component sum over j in 2 adds: (j0+j2, j1+j3) then pair-sum
                U_sb = upool.tile([128, 2048], BF16)
                Uv = U_sb.rearrange("p (e g c) -> p e g c", e=EPG, g=2)
                v.tensor_add(Uv[:], Tv[:, :, 0:2, :], Tv[:, :, 2:4, :])
                S_sb = spool.tile([128, 1024], BF16)
                Sv = S_sb.rearrange("p (e c) -> p e c", e=EPG)
                v.tensor_add(Sv[:], Uv[:, :, 0, :], Uv[:, :, 1, :])
                st_ps = [ps_st.tile([128, 512], BF16, name=f"stps{h}", tag=f"stps{h}")
                         for h in range(2)]
                for e in range(EPG):
                    for h in range(2):
                        nc.tensor.transpose(
                            st_ps[h][:, e * 128:(e + 1) * 128],
                            S_sb[:, e * C + h * 128: e * C + (h + 1) * 128],
                            ident_bf[:])
                st_sb = [stpool.tile([128, 512], BF16, name=f"st{h}", tag=f"st{h}")
                         for h in range(2)]
                stsq = [stpool.tile([128, 512], BF16, name=f"sq{h}", tag=f"sq{h}")
                        for h in range(2)]
                for h in range(2):
                    v.tensor_copy(st_sb[h][:], st_ps[h][:])
                    a.activation(stsq[h][:], st_ps[h][:], AF.Square)
                # P_T[n, b] = S_n . A_b with n on partitions: out-partition of
                # block e is the original bank partition p, so rh4[:, e] is a
                # legal per-partition scale for the exp.  ssq rides in the same
                # PSUM bank (cols 256:260), reduced over c by PE ones-matmuls.
                p_all = ps_p.tile([128, EPG * B + EPG], F32)
                for e in range(EPG):
                    for h in range(2):
                        nc.tensor.matmul(p_all[:, e * B:(e + 1) * B],
                                         st_sb[h][:, e * 128:(e + 1) * 128],
                                         at_sb[:, h, :],
                                         start=(h == 0), stop=(h == 1))
                for e in range(EPG):
                    for h in range(2):
                        nc.tensor.matmul(
                            p_all[:, EPG * B + e:EPG * B + e + 1],
                            stsq[h][:, e * 128:(e + 1) * 128], ones_bf[:],
                            start=(h == 0), stop=(h == 1))
                # rh4 = 0.5/||S||  (the ln(1/2) bias folds in the 1/tau factor)
                rh4 = sqpool.tile([128, EPG], F32, tag="rh4")
                a.activation(rh4[:], p_all[:, EPG * B:EPG * B + EPG], AF.Ln)
                a.activation(rh4[:], rh4[:], AF.Exp, scale=-0.5, bias=lnhalf_sb[:])
                if pend is not None:
                    _emit_exp_stage(*pend)
                pend = (p_all, rh4, k, first, last)
            _emit_exp_stage(*pend)

            # ---------- epilogue: partial scalars ----------
            E_row = small.tile([1, K * B], F32)
            a.copy(E_row[:], E_all[:, 0:K * B])
            nc.sync.dma_start(out=e_dbg[:], in_=E_row[:])
            epm = small.tile([1, K * B], F32)
            ep1 = small.tile([1, 1], F32)
            rsum = small.tile([1, 1], F32)
            en1 = small.tile([1, 1], F32)
            v.scalar_tensor_tensor(epm[:], E_row[:], 1.0, oh4t_sb[:],
                                   op0=mybir.AluOpType.bypass,
                                   op1=mybir.AluOpType.mult, accum_out=ep1[:])
            v.reduce_sum(rsum[:], E_row[:], axis=mybir.AxisListType.X)
            v.tensor_scalar_mul(en1[:], ep1[:], -1.0)
            v.tensor_add(en1[:], en1[:], rsum[:])
            # contrib sum over b via PE ones-reduction
            nc.tensor.matmul(cs_ps[:], contrib[:], ones_sb[0:B, :],
                             start=True, stop=True, skip_group_check=True)
            F1 = small.tile([1, 4], F32)
            v.memset(F1[:], 0.0)
            v.tensor_scalar_mul(F1[:, 0:1], ep1[:], 1.0 / (B * NB))
            v.tensor_scalar_mul(F1[:, 1:2], en1[:], 1.0 / (B * (K - 1) * NB))
            v.tensor_copy(F1[:, 2:3], cs_ps[:])
            nc.sync.dma_start(out=out_d[:], in_=F1[:])

    nc.compile()
    return nc


_NC = None


def _get_nc():
    global _NC
    if _NC is None:
        _NC = _build()
    return _NC


def _make_in_maps(hazards, S, indiv, gene, path, cohort_bank, label, c):
    hazards = np.asarray(hazards, dtype=np.float32)
    S = np.asarray(S, dtype=np.float32)
    indiv = np.asarray(indiv, dtype=np.float32)
    gene = np.asarray(gene, dtype=np.float32)
    path = np.asarray(path, dtype=np.float32)
    cohort_bank = np.asarray(cohort_bank, dtype=np.float32)
    label = np.asarray(label)
    c = np.asarray(c)

    blob = np.zeros((128, 1824), np.float32)
    blob[0:B, 0:1024] = indiv.reshape(B, -1)
    blob[0:B, 1024:1536] = np.concatenate(
        [gene.reshape(B, -1), path.reshape(B, -1)], axis=1)
    blob[0:B, 1536:1540] = hazards
    blob[0:B, 1540:1545] = np.concatenate(
        [np.ones((B, 1), np.float32), S], axis=1)
    oh5 = np.zeros((B, K + 1), np.float32)
    oh5[np.arange(B), label] = 1.0
    blob[0:B, 1545:1550] = oh5
    oh5b = np.zeros((B, K + 1), np.float32)
    oh5b[np.arange(B), label + 1] = 1.0
    blob[0:B, 1550:1555] = oh5b
    blob[0:B, 1555] = c.astype(np.float32)
    blob[0:B, 1556] = 1.0 - c.astype(np.float32)
    blob[0, 1557 + label.astype(np.int64) * B + np.arange(B)] = 1.0
    common = dict(blob=blob)
    bankf = cohort_bank.reshape(K, NB, 1024)
    in_maps = []
    for i in range(NCORES):
        shard = np.ascontiguousarray(
            bankf[:, i * NSH:(i + 1) * NSH, :]).reshape(ROWS, 1024)
        in_maps.append({**common, "bank": shard})
    return in_maps


_LAST_RESULTS = None  # stashed for test.py introspection


def kernel(hazards, S, indiv, gene, path, cohort_bank, label, c):
    global _LAST_RESULTS
    nc = _get_nc()
    in_maps = _make_in_maps(hazards, S, indiv, gene, path, cohort_bank, label, c)
    trace = bool(int(os.environ.get("TRNK_TRACE", "0")))
    res = run_bass_kernel_spmd(nc, in_maps, core_ids=list(range(NCORES)),
                               trace=trace)
    _LAST_RESULTS = res
    outs = np.stack([r["out_vec"][0, :] for r in res.results])  # [8, 4]
    ep = float(outs[:, 0].sum())
    en = float(outs[:, 1].sum())
    other = float(outs[:, 2].mean())
    loss = other - math.log((ep + EPS_COH) / (ep + en + EPS_COH))
    return np.float32(loss)
